# revision 10
# baseline (speedup 1.0000x reference)
"""GATv2 3-layer GNN on 8 TRN2 NeuronCores (v2).

Sharding: edges sorted by destination node; dst-range sharded across the 8
cores (1250 nodes per core).  Per layer each core computes xl/xr for its own
node slice, the xl slices are all-gathered (bf16) so every core can gather
xl[src] for arbitrary src, and each core aggregates messages for its dst
range only (no cross-core reduction needed).

v2 changes vs v1:
  - one-hot matrices (comb = [ohT ; eaT] and oh) precomputed on host and
    streamed from DRAM per block -- no on-device is_equal/transpose/copies.
  - We is written once per layer into rows 119:127 of the xr slice tile, so
    the m matmul's moving operand is just xr_sl[:, b, :] (no per-block
    rhs assembly).
  - logits via 8 per-head tensor_tensor_reduce ops (fused mult+reduce),
    exp batched per 8-group chunk, message weighting via per-head
    tensor_scalar with the per-partition scalar ex[:, g*8+h].
  - gathers issued as prepare_only + trigger_dma so SDMA transfers queue
    back-to-back instead of stalling the Q7 per call.
  - chunk-skewed software pipeline: m/prelu/logits of chunk c overlap
    w/acc/den of chunk c-1.
"""

import os
import numpy as np
import ml_dtypes

# ---- problem constants (hardcoded per spec nn_GATv2_5454608466160) ----
N = 10000
E = 160000
F_NODE = 16
F_EDGE = 8
H = 8
C = 64
HC = 512
NCLS = 4
SLOPE = 0.2
EPS = 1e-16

NCORES = 8
SLICE = 1250          # real nodes owned per core
BLK = 119             # dst nodes per block (119 + 8 edge-attr rows + 1 = 128)
NBLK = 11             # blocks per core (11*119 = 1309 >= 1250)
SLICE_PAD = BLK * NBLK
NPAD = NCORES * SLICE_PAD  # rows in the gathered xl table

CH = 8                # groups per gather/exp chunk

BF16 = ml_dtypes.bfloat16

_cache = {}


# --------------------------------------------------------------------------
# host-side preprocessing
# --------------------------------------------------------------------------
def _prep_edges(edge_index, edge_attr):
    """Sort edges by dst, shard by dst range, block by BLK dst nodes,
    pad each (core, block) to a common group count; build the one-hot
    matmul operands on the host."""
    src = edge_index[0].astype(np.int64)
    dst = edge_index[1].astype(np.int64)
    order = np.argsort(dst, kind="stable")
    src, dst = src[order], dst[order]
    ea = edge_attr[order]

    core_of = dst // SLICE
    blk_of = (dst - core_of * SLICE) // BLK
    counts = np.zeros((NCORES, NBLK), dtype=np.int64)
    for k in range(NCORES):
        m = core_of == k
        counts[k] = np.bincount(blk_of[m], minlength=NBLK)
    G = np.maximum(1, np.ceil(counts.max(axis=0) / 128).astype(np.int64))
    e_pad = int(G.sum() * 128)
    g_tot = e_pad // 128

    src_pos = np.zeros((NCORES, e_pad), dtype=np.int64)
    dst_loc = np.full((NCORES, e_pad), -1, dtype=np.int64)
    ea_pad = np.zeros((NCORES, e_pad, F_EDGE), dtype=np.float32)
    boff = np.concatenate([[0], np.cumsum(G)])  # group offset of each block

    for k in range(NCORES):
        mk = core_of == k
        sk, dk, eak, bk = src[mk], dst[mk], ea[mk], blk_of[mk]
        for b in range(NBLK):
            mb = bk == b
            n = int(mb.sum())
            o = int(boff[b]) * 128
            src_pos[k, o:o + n] = sk[mb]
            dst_loc[k, o:o + n] = dk[mb] - k * SLICE - b * BLK
            ea_pad[k, o:o + n] = eak[mb]

    # src -> row position in the all-gathered xl table
    s_slice = src_pos // SLICE
    s_local = src_pos - s_slice * SLICE
    gpos = (s_slice * SLICE_PAD + s_local).astype(np.int16)

    # dma_gather index wrap: idx i at [i % 16, i // 16], tiled to 128 parts
    gidx = np.ascontiguousarray(
        gpos.reshape(NCORES, e_pad // 16, 16).transpose(0, 2, 1))
    gidx = np.tile(gidx, (1, 8, 1))                      # [NCORES,128,e_pad//16]

    # comb [NCORES, 128, e_pad]: per group g the [128,128] tile
    # comb[:, :, g*128:(g+1)*128] has rows 0:119 = one-hot(dst within block),
    # rows 119:127 = edge_attr^T, row 127 = 0.  (lhsT of the m matmul)
    comb = np.zeros((NCORES, 128, e_pad), dtype=np.float32)
    for k in range(NCORES):
        valid = dst_loc[k] >= 0
        cols = np.nonzero(valid)[0]
        comb[k, dst_loc[k, valid], cols] = 1.0
        comb[k, BLK:BLK + F_EDGE, :] = ea_pad[k].T
    comb = comb.astype(BF16)

    # oh [NCORES, 128, g_tot, 128]: oh[p, g, n] = dst_loc[g*128+p] == n
    dstr = dst_loc.reshape(NCORES, g_tot, 128)
    oh = (dstr[:, :, :, None] == np.arange(128)[None, None, None, :])
    oh = np.ascontiguousarray(
        oh.transpose(0, 2, 1, 3)).astype(np.float32).astype(BF16)

    return {
        "G": tuple(int(g) for g in G),
        "e_pad": e_pad,
        "gidx": gidx,
        "comb": comb,
        "oh": oh,
    }


def _to_bf16(x):
    return np.asarray(x, dtype=np.float32).astype(BF16)


# --------------------------------------------------------------------------
# device kernel build
# --------------------------------------------------------------------------
def _build(G, e_pad, nonzero_bias):
    import concourse.bass as bass
    import concourse.bacc as bacc
    import concourse.mybir as mybir
    import concourse.tile as tile
    from concourse import library_config

    f32 = mybir.dt.float32
    bf16 = mybir.dt.bfloat16
    i16 = mybir.dt.int16
    AF = mybir.ActivationFunctionType
    OP = mybir.AluOpType

    g_tot = e_pad // 128
    boff = [0]
    for g in G:
        boff.append(boff[-1] + g)

    nc = bacc.Bacc("TRN2", target_bir_lowering=False, debug=False,
                   num_devices=NCORES)

    # ---- I/O ----
    def inp(name, shape, dt=bf16):
        return nc.dram_tensor(name, shape, dt, kind="ExternalInput")

    xT0 = inp("xT0", [F_NODE, SLICE_PAD])
    Wls = [inp(f"Wl{l}", [F_NODE if l == 0 else HC, HC]) for l in range(3)]
    Wrs = [inp(f"Wr{l}", [F_NODE if l == 0 else HC, HC]) for l in range(3)]
    WeBs = [inp(f"WeB{l}", [F_EDGE + 1, NBLK, HC]) for l in range(3)]
    attBs = [inp(f"attB{l}", [128, HC]) for l in range(3)]
    Wf = inp("Wf", [HC, NCLS])
    comb_h = inp("comb", [128, e_pad])
    oh_h = inp("oh", [128, g_tot, 128])
    gidx_h = inp("gidx", [128, e_pad // 16], i16)
    I128_h = inp("ident", [128, 128])
    biasB = None
    if nonzero_bias:
        biasB = {
            "lr": [inp(f"blrB{l}", [128, HC]) for l in range(3)],
            "rr": [inp(f"brrB{l}", [128, HC]) for l in range(3)],
            "bo": [inp(f"boB{l}", [128, HC]) for l in range(3)],
            "bf": inp("bfB", [128, NCLS]),
        }

    out_h = nc.dram_tensor("out", [SLICE_PAD, NCLS], f32, kind="ExternalOutput")

    use_prep = os.environ.get("KPREP", "1") == "1"
    use_ttr = os.environ.get("KTTR", "1") == "1"

    with tile.TileContext(nc) as tc:
        import contextlib
        ctx = contextlib.ExitStack()
        with ctx:
            cpool = ctx.enter_context(tc.tile_pool(name="const", bufs=1))
            wpool = ctx.enter_context(tc.tile_pool(name="weights", bufs=1))
            hpool = ctx.enter_context(tc.tile_pool(name="hT", bufs=1))
            spool = ctx.enter_context(tc.tile_pool(name="slices", bufs=1))
            gpool = ctx.enter_context(tc.tile_pool(name="gather", bufs=4))
            opool = ctx.enter_context(tc.tile_pool(name="ohc", bufs=2))
            mpool = ctx.enter_context(tc.tile_pool(name="mact", bufs=6))
            wvpool = ctx.enter_context(tc.tile_pool(name="wv", bufs=4))
            scpool = ctx.enter_context(tc.tile_pool(name="scratch", bufs=3))
            expool = ctx.enter_context(tc.tile_pool(name="expb", bufs=3))
            dpool = ctx.enter_context(tc.tile_pool(name="dram", bufs=1,
                                                   space="DRAM"))
            ps_m = ctx.enter_context(tc.tile_pool(name="ps_m", bufs=3,
                                                  space="PSUM"))
            ps_o = ctx.enter_context(tc.tile_pool(name="ps_o", bufs=2,
                                                  space="PSUM"))
            ps_d = ctx.enter_context(tc.tile_pool(name="ps_d", bufs=1,
                                                  space="PSUM"))
            ps_t = ctx.enter_context(tc.tile_pool(name="ps_t", bufs=1,
                                                  space="PSUM"))

            nc.gpsimd.load_library(library_config.mlp)
            dma_sem = nc.alloc_semaphore("swdge_dma") if use_prep else None

            # ---- load constants ----
            _ldn = [0]
            def load(pool, src_ap, shape, dt=bf16, tag=None):
                _ldn[0] += 1
                t = pool.tile(shape, dt, tag=tag or f"cst{_ldn[0]}")
                nc.sync.dma_start(t[:], src_ap)
                return t

            I128_sb = load(cpool, I128_h[:, :], [128, 128])
            attB_sb = [load(cpool, attBs[l][:, :], [128, HC]) for l in range(3)]
            gidx_sb = load(cpool, gidx_h[:, :], [128, e_pad // 16], i16)
            Wf_sb = load(cpool, Wf.ap().rearrange("(kc p) n -> p kc n", p=128),
                         [128, 4, NCLS])
            bias_sb = None
            if nonzero_bias:
                bias_sb = {
                    "lr": [load(cpool, biasB["lr"][l][:, :], [128, HC])
                           for l in range(3)],
                    "rr": [load(cpool, biasB["rr"][l][:, :], [128, HC])
                           for l in range(3)],
                    "bo": [load(cpool, biasB["bo"][l][:, :], [128, HC])
                           for l in range(3)],
                    "bf": load(cpool, biasB["bf"][:, :], [128, NCLS]),
                }

            Wl_sb, Wr_sb = [], []
            for l in range(3):
                if l == 0:
                    Wl_sb.append(load(wpool, Wls[0][:, :], [F_NODE, HC]))
                    Wr_sb.append(load(wpool, Wrs[0][:, :], [F_NODE, HC]))
                else:
                    Wl_sb.append(load(
                        wpool, Wls[l].ap().rearrange("(kc p) n -> p kc n", p=128),
                        [128, 4, HC]))
                    Wr_sb.append(load(
                        wpool, Wrs[l].ap().rearrange("(kc p) n -> p kc n", p=128),
                        [128, 4, HC]))

            xT0_sb = load(hpool, xT0[:, :], [F_NODE, SLICE_PAD])

            hT = None  # [128, 4, SLICE_PAD] bf16 for layers >= 1

            for l in range(3):
                KCH = 1 if l == 0 else 4

                # ---------------- node phase ----------------
                xl_sl = spool.tile([128, NBLK, HC], bf16, tag="xl_sl")
                xr_sl = spool.tile([128, NBLK, HC], bf16, tag="xr_sl")
                for cblk in range(NBLK):
                    for which, Wsb, dst_t in (("l", Wl_sb[l], xl_sl),
                                              ("r", Wr_sb[l], xr_sl)):
                        ps = ps_m.tile([128, HC], f32, tag="m_ps")
                        for kc in range(KCH):
                            if l == 0:
                                lhsT = xT0_sb[:, cblk * BLK:(cblk + 1) * BLK]
                                rhs = Wsb[:, :]
                            else:
                                lhsT = hT[:, kc, cblk * BLK:(cblk + 1) * BLK]
                                rhs = Wsb[:, kc, :]
                            nc.tensor.matmul(ps[0:BLK, :], lhsT, rhs,
                                             start=(kc == 0),
                                             stop=(kc == KCH - 1))
                        if nonzero_bias:
                            b = bias_sb["lr" if which == "l" else "rr"][l]
                            nc.vector.tensor_tensor(
                                out=dst_t[0:BLK, cblk, :], in0=ps[0:BLK, :],
                                in1=b[0:BLK, :], op=OP.add)
                        else:
                            nc.scalar.activation(dst_t[0:BLK, cblk, :],
                                                 ps[0:BLK, :], AF.Copy)
                # We of this layer into rows 119:127 of every block column
                # (plus a zero row 127): the m matmul's moving operand is
                # then just xr_sl[:, b, :].
                nc.sync.dma_start(xr_sl[BLK:128, :, :], WeBs[l][:, :, :])

                # ---------------- all-gather xl ----------------
                xl_dram = dpool.tile([SLICE_PAD, HC], bf16, tag="xl_dram")
                xl_full = dpool.tile([NPAD, HC], bf16, tag="xl_full")
                nc.sync.dma_start(
                    xl_dram[:].rearrange("(s p) n -> p s n", p=BLK),
                    xl_sl[0:BLK, :, :])
                nc.gpsimd.collective_compute(
                    "AllGather",
                    mybir.AluOpType.bypass,
                    replica_groups=[list(range(NCORES))],
                    ins=[xl_dram[:].opt()],
                    outs=[xl_full[:].opt()],
                )

                # ---------------- edge phase ----------------
                hT_next = hpool.tile([128, 4, SLICE_PAD], bf16, tag="hT_next")
                for b in range(NBLK):
                    Gb = G[b]
                    # stream this block's one-hot operands from DRAM
                    comb_sb = opool.tile([128, 21 * 128], bf16, tag="comb")
                    nc.sync.dma_start(
                        comb_sb[:, 0:Gb * 128],
                        comb_h[:, boff[b] * 128:(boff[b] + Gb) * 128])
                    oh_sb = opool.tile([128, 21, 128], bf16, tag="oh")
                    nc.sync.dma_start(oh_sb[:, 0:Gb, :],
                                      oh_h[:, boff[b]:boff[b] + Gb, :])

                    acc = ps_o.tile([128, HC], f32, tag="acc")
                    den = ps_d.tile([128, F_EDGE], f32, tag="den")
                    chunks = [(c0, min(CH, Gb - c0)) for c0 in range(0, Gb, CH)]

                    # gather preps + triggers for all chunks of the block;
                    # transfers queue up behind the all-gather dependency.
                    xgs = {}
                    for (c0, cn) in chunks:
                        xg = gpool.tile([128, CH, HC], bf16, tag="xg")
                        if use_prep:
                            nc.gpsimd.dma_gather(
                                xg[:, 0:cn, :], xl_full[:],
                                gidx_sb[:, (boff[b] + c0) * 8:
                                        (boff[b] + c0 + cn) * 8],
                                cn * 128, cn * 128, HC,
                                prepare_only=True)
                            nc.gpsimd.trigger_dma(count=None)
                        else:
                            nc.gpsimd.dma_gather(
                                xg[:, 0:cn, :], xl_full[:],
                                gidx_sb[:, (boff[b] + c0) * 8:
                                        (boff[b] + c0 + cn) * 8],
                                cn * 128, cn * 128, HC)
                        xgs[c0] = xg

                    # chunk-skewed pipeline: phase A of chunk c, then
                    # phase B of chunk c-1.
                    stash = {}

                    def phase_a(c0, cn):
                        xg = xgs[c0]
                        elog = expool.tile([128, CH * H], f32, tag="elog")
                        mas = []
                        for j in range(cn):
                            g = c0 + j
                            m_ps = ps_m.tile([128, HC], f32, tag="m_ps")
                            nc.tensor.matmul(
                                m_ps[:],
                                comb_sb[:, g * 128:(g + 1) * 128],
                                xr_sl[:, b, :], start=True, stop=False)
                            nc.tensor.matmul(m_ps[:], I128_sb[:],
                                             xg[:, j, :],
                                             start=False, stop=True)
                            ma = mpool.tile([128, HC], bf16, tag="ma")
                            nc.scalar.activation(ma[:], m_ps[:], AF.Prelu,
                                                 alpha=SLOPE)
                            mas.append(ma)
                            if use_ttr:
                                sc = scpool.tile([128, C], bf16, tag="scr")
                                for h in range(H):
                                    nc.vector.tensor_tensor_reduce(
                                        out=sc[:],
                                        in0=ma[:, h * C:(h + 1) * C],
                                        in1=attB_sb[l][:, h * C:(h + 1) * C],
                                        scale=1.0, scalar=0.0,
                                        op0=OP.mult, op1=OP.add,
                                        accum_out=elog[:, j * H + h:
                                                       j * H + h + 1])
                            else:
                                m2 = scpool.tile([128, HC], bf16, tag="m2")
                                nc.vector.tensor_tensor(
                                    out=m2[:], in0=ma[:], in1=attB_sb[l][:],
                                    op=OP.mult)
                                nc.vector.tensor_reduce(
                                    elog[:, j * H:(j + 1) * H],
                                    m2[:].rearrange("p (h c) -> p h c", c=C),
                                    axis=mybir.AxisListType.X, op=OP.add)
                        exf = expool.tile([128, CH * H], f32, tag="exf")
                        nc.scalar.activation(exf[:, 0:cn * H],
                                             elog[:, 0:cn * H], AF.Exp)
                        exb = expool.tile([128, CH * H], bf16, tag="exb")
                        nc.scalar.activation(exb[:, 0:cn * H],
                                             exf[:, 0:cn * H], AF.Copy)
                        stash[c0] = (exf, exb)

                    def phase_b(c0, cn, last):
                        xg = xgs[c0]
                        exf, exb = stash.pop(c0)
                        for j in range(cn):
                            g = c0 + j
                            w = wvpool.tile([128, HC], bf16, tag="w")
                            for h in range(H):
                                nc.vector.tensor_scalar(
                                    out=w[:, h * C:(h + 1) * C],
                                    in0=xg[:, j, h * C:(h + 1) * C],
                                    scalar1=exf[:, j * H + h:j * H + h + 1],
                                    scalar2=None, op0=OP.mult)
                            glast = last and (j == cn - 1)
                            nc.tensor.matmul(acc[:],
                                             oh_sb[:, g, :], w[:],
                                             start=(g == 0), stop=glast)
                            nc.tensor.matmul(den[:],
                                             oh_sb[:, g, :],
                                             exb[:, j * H:(j + 1) * H],
                                             start=(g == 0), stop=glast)

                    for ci, (c0, cn) in enumerate(chunks):
                        phase_a(c0, cn)
                        if ci > 0:
                            p0, pn = chunks[ci - 1]
                            phase_b(p0, pn, last=False)
                    phase_b(*chunks[-1], last=True)

                    # ---- block epilogue ----
                    dinv = scpool.tile([128, H], f32, tag="dinv")
                    nc.vector.tensor_scalar(out=dinv[:], in0=den[:],
                                            scalar1=EPS, scalar2=None,
                                            op0=OP.add)
                    nc.vector.reciprocal(dinv[:], dinv[:])
                    hpre = mpool.tile([128, HC], bf16, tag="hpre")
                    nc.vector.tensor_tensor(
                        out=hpre[:].rearrange("p (h c) -> p h c", c=C),
                        in0=acc[:].rearrange("p (h c) -> p h c", c=C),
                        in1=dinv[:].unsqueeze(2).broadcast_to([128, H, C]),
                        op=OP.mult)
                    if nonzero_bias:
                        nc.vector.tensor_tensor(out=hpre[:], in0=hpre[:],
                                                in1=bias_sb["bo"][l][:],
                                                op=OP.add)
                    if l < 2:
                        for fc in range(4):
                            tr = ps_t.tile([128, 128], bf16, tag="trp")
                            nc.tensor.transpose(
                                tr[:], hpre[:, fc * 128:(fc + 1) * 128],
                                I128_sb[:])
                            nc.scalar.activation(
                                hT_next[:, fc, b * BLK:(b + 1) * BLK],
                                tr[:, 0:BLK], AF.Prelu, alpha=SLOPE)
                    else:
                        # final layer: classifier on lrelu(h3)
                        h3 = mpool.tile([128, HC], bf16, tag="h3")
                        nc.scalar.activation(h3[:], hpre[:], AF.Prelu,
                                             alpha=SLOPE)
                        ops = ps_m.tile([128, NCLS], f32, tag="m_ps")
                        for fc in range(4):
                            tr = ps_t.tile([128, 128], bf16, tag="trp")
                            nc.tensor.transpose(
                                tr[:], h3[:, fc * 128:(fc + 1) * 128],
                                I128_sb[:])
                            h3T = scpool.tile([128, 128], bf16, tag="h3T")
                            nc.scalar.activation(h3T[:], tr[:], AF.Copy)
                            nc.tensor.matmul(ops[:], h3T[:], Wf_sb[:, fc, :],
                                             start=(fc == 0), stop=(fc == 3))
                        osb = mpool.tile([128, NCLS], f32, tag="osb")
                        if nonzero_bias:
                            nc.vector.tensor_tensor(out=osb[:], in0=ops[:],
                                                    in1=bias_sb["bf"][:],
                                                    op=OP.add)
                        else:
                            nc.scalar.activation(osb[:], ops[:], AF.Copy)
                        nc.sync.dma_start(
                            out_h.ap().rearrange("(s p) n -> p s n", p=BLK)
                            [:, b, :], osb[0:BLK, :])
                if l < 2:
                    hT = hT_next

    nc.compile()
    return nc


# --------------------------------------------------------------------------
# public entry point
# --------------------------------------------------------------------------
def _get_compiled(inputs):
    prep = _prep_edges(inputs["edge_index"], inputs["edge_attr"])
    nonzero_bias = any(
        np.abs(inputs[k]).max() > 0
        for k in ("bl0", "br0", "bo0", "bl1", "br1", "bo1",
                  "bl2", "br2", "bo2", "bf"))
    key = (prep["G"], prep["e_pad"], nonzero_bias)
    if key not in _cache:
        _cache[key] = _build(prep["G"], prep["e_pad"], nonzero_bias)
    return _cache[key], prep, nonzero_bias


def _make_in_maps(inputs, prep, nonzero_bias):
    x = np.asarray(inputs["x"], dtype=np.float32)
    xpad = np.zeros((NCORES, SLICE_PAD, F_NODE), dtype=np.float32)
    xr = x.reshape(NCORES, SLICE, F_NODE)
    xpad[:, :SLICE] = xr
    ident = np.eye(128, dtype=np.float32).astype(BF16)

    common = {
        "ident": ident,
        "Wf": _to_bf16(inputs["Wf"]),
    }
    for l in range(3):
        common[f"Wl{l}"] = _to_bf16(inputs[f"Wl{l}"])
        common[f"Wr{l}"] = _to_bf16(inputs[f"Wr{l}"])
        WeP = np.zeros((F_EDGE + 1, HC), dtype=np.float32)
        WeP[:F_EDGE] = np.asarray(inputs[f"We{l}"], np.float32)
        common[f"WeB{l}"] = _to_bf16(
            np.broadcast_to(WeP[:, None, :], (F_EDGE + 1, NBLK, HC)).copy())
        att = np.asarray(inputs[f"att{l}"], np.float32).reshape(1, HC)
        common[f"attB{l}"] = _to_bf16(np.tile(att, (128, 1)))
    if nonzero_bias:
        for l in range(3):
            common[f"blrB{l}"] = _to_bf16(
                np.tile(np.asarray(inputs[f"bl{l}"]).reshape(1, HC), (128, 1)))
            common[f"brrB{l}"] = _to_bf16(
                np.tile(np.asarray(inputs[f"br{l}"]).reshape(1, HC), (128, 1)))
            common[f"boB{l}"] = _to_bf16(
                np.tile(np.asarray(inputs[f"bo{l}"]).reshape(1, HC), (128, 1)))
        common["bfB"] = _to_bf16(
            np.tile(np.asarray(inputs["bf"]).reshape(1, NCLS), (128, 1)))

    in_maps = []
    for k in range(NCORES):
        m = dict(common)
        m["xT0"] = np.ascontiguousarray(xpad[k].T).astype(BF16)
        m["comb"] = prep["comb"][k]
        m["oh"] = prep["oh"][k]
        m["gidx"] = prep["gidx"][k]
        in_maps.append(m)
    return in_maps


def run(inputs, trace=False, **kw):
    from concourse.bass_utils import run_bass_kernel_spmd
    nc, prep, nonzero_bias = _get_compiled(inputs)
    in_maps = _make_in_maps(inputs, prep, nonzero_bias)
    res = run_bass_kernel_spmd(nc, in_maps, core_ids=list(range(NCORES)),
                               trace=trace, **kw)
    outs = [res.results[k]["out"][:SLICE] for k in range(NCORES)]
    full = np.concatenate(outs, axis=0).astype(np.float32)
    return full, res


def kernel(**inputs):
    out, _ = run(inputs, trace=False)
    return out
